# revision 30
# baseline (speedup 1.0000x reference)
"""MinibatchDiscrimination Trainium2 kernel (8-core SPMD, Bass/Tile).

Reference computation:
    m   = einsum('bf,fkd->bkd', x, kernel)        # B=512, F=512, K=128, D=16
    l1  = sum_d |m[i,k,d] - m[j,k,d]|             # [B, B, K]
    mb  = sum_j exp(-l1)                          # [B, K]
    out = concat([x, mb], axis=1)                 # [B, F+K]

Sharding: symmetric circulant row parallelism. Device d owns rows
R_d = [64d, 64d+64) and a wrapped column window W_d = [64d, 64d+320).
Every (i in R_d, j in W_d) pair is computed once; rows get their
"far" (block-distance 5..7) contributions from other devices' j-side
partial sums, using l1 symmetry. Coverage: for an unordered block pair
at distance delta, delta in {0..4} is computed by the lower block
(i-side), delta in {1,2,3} additionally produces j-side partials that
serve as the distance {7,6,5} contributions of the other block.
Distance 4 is computed by both sides (i-side only), so the j-side only
spans window columns [64, 256).

Per-core dataflow:
  mT[kd, w] = kern.T @ xT  (PE, bf16)            # [2048, 320] as 16 kg-tiles
  for each of 64 rows i, 16 kg:  |mT - mT[:, i]| via a custom DVE op
    (out = max(Src0-C0, C0-Src0)) or ACT Abs with per-partition bias;
    PE selector-matmul reduces d (16 partitions per k) into PSUM l1.
  ACT exp(-l1) emits e (bf16) + accum_out = i-side row sums.
  PE ones-matmul sums e over the 16 i's -> j-side partials.
"""

import numpy as np
import ml_dtypes

import concourse.bacc as bacc
import concourse.bass as bass  # noqa: F401
import concourse.tile as tile
import concourse.mybir as mybir
import concourse.dve_ops as dve_ops
from concourse.dve_ops import DveOp
from concourse.dve_spec import Spec, Src0, C0, Bin
from concourse.dve_uop import (
    UopConfig, UopDpConfig, AluOp, AluInp, DelayInp, InpSel, OutSel, OutPath,
    Trigger, DveOpSpec,
)
from concourse.dve_tables import load_table_set, find_stock_dve_bin_dir
from concourse.bass_utils import run_bass_kernel_spmd

B, F, K, D = 512, 512, 128, 16
NC = 8          # cores
MY = 64         # rows per core
W = 320         # window columns per core
JS0, JS1 = 64, 256   # j-side window column range
KG = 16         # k-groups (8 k each, x16 d = 128 partitions)
IB = 4          # i-blocks of 16 rows
N_DVE = 15      # absdiffs per group assigned to DVE (rest to ACT)

bf16 = mybir.dt.bfloat16
f32 = mybir.dt.float32
AF = mybir.ActivationFunctionType


def _dp_from_entry(e: dict) -> UopDpConfig:
    # Substitute the stock program's instruction-indirected ops with
    # concrete ones: INSTRUCTION_OP_0 -> ABSOLUTE_DIFF, _1 -> BYPASS
    # (BYPASS forwards PREV_ALU_OUT, keeping all routing identical).
    alu = e.get("alu_op", 0)
    if alu == 32:
        op = AluOp.ABSOLUTE_DIFF
    elif alu == 33:
        op = AluOp.BYPASS
    else:
        op = AluOp(alu)
    return UopDpConfig(
        op=op,
        alu_src0=AluInp(e.get("mux0_sel", 0)),
        alu_src1=AluInp(e.get("mux1_sel", 0)),
        delay=[DelayInp(e.get(f"d{i}_sel", 0)) for i in range(7)],
        alu_out_enable=e.get("out_flop_enable", 0),
        swap_enable=e.get("swap_flop_enable", 0),
        alu_out_a_enable=e.get("out_a_flop_enable", 0),
        alu_out_b_enable=e.get("out_b_flop_enable", 0),
        delay_enable=[e.get(f"d{i}_flop_enable", 0) for i in range(7)],
    )


def _uop_from_slot(ts, slot: int) -> UopConfig:
    cf, cs, dp = ts.control_fast[slot], ts.control_slow[slot], ts.datapath[slot]
    en = cs.get("input_enable", 0)
    selmap = {
        OutPath.WR0_LO: ("write0_sel_lo", "write0_en_lo"),
        OutPath.WR0_HI: ("write0_sel_hi", "write0_en_hi"),
        OutPath.WR1_LO: ("write1_sel_lo", "write1_en_lo"),
        OutPath.WR1_HI: ("write1_sel_hi", "write1_en_hi"),
    }
    return UopConfig(
        inp=[InpSel(cs.get(f"inp{i}", 0)) for i in range(8)],
        inp_enable=[(en >> i) & 1 for i in range(8)],
        out={p: OutSel(cs.get(sk, 0)) for p, (sk, _) in selmap.items()},
        out_enable={p: cf.get(ek, 0) for p, (_, ek) in selmap.items()},
        require_inp0=cf.get("requires_src0", 0),
        require_inp1=cf.get("requires_src1", 0),
        trigger=(Trigger(cf.get("trigger0", 0)), Trigger.NONE, Trigger.NONE),
        next_uop=(0, 0, 0),
        enable_rev_ops=0,
        datapath_config=[_dp_from_entry(e) for e in dp],
    )


def _register_absdiff() -> DveOp:
    """out = |in0 - s0| as one ABSOLUTE_DIFF stage, with 2x/2x_2p/4x
    perf-mode uop programs cloned from the stock gen3
    TENSOR_SCALAR_PTR_ARITH_OP table (opcode 68)."""
    name = "ABSDIFF2_ANT"
    for op in dve_ops.OPS:
        if op.name == name:
            return op
    spec = Spec(
        body=Bin(AluOp.ABSOLUTE_DIFF, Src0, C0),
        reference=lambda in0, in1, s0, s1, imm2: np.abs(
            np.asarray(in0, np.float32) - s0
        ),
    )
    row = dve_ops._CUSTOM_DVE_ROW_BASE + len(dve_ops.OPS)
    assert row < 0x20, "no free custom-DVE rows"

    ts = load_table_set(find_stock_dve_bin_dir("gen3"), "default", "v3")
    base = ts.opcode[68]["table_ptr"]
    uops = [_uop_from_slot(ts, base + m) for m in range(4)]
    for u in uops:
        u.validate("v3")
    dspec = DveOpSpec(name=name, opcode=row, uops=[uops[0]],
                      uops_2x=[uops[1]], uops_2x_2p=[uops[2]],
                      uops_4x=[uops[3]], rd1_en=False)

    class _FixedDveOp(DveOp):
        def compile(self, ver):
            assert ver == "v3", f"{name} only authored for v3, got {ver}"
            return dspec

    op = _FixedDveOp(name, spec, subdim=False, uops_sha={})
    dve_ops.OPS.append(op)
    dve_ops._SUB_OPCODE_FOR_NAME[name] = row
    dve_ops.CUSTOM_DVE_SPECS[name] = spec
    return op


def build_module(n_dve: int = N_DVE, pe_filler: int = 0, small_pe: bool = False,
                 ad_bufs: int = 8, e_bufs: int = 4, l1_bufs: int = 4,
                 kg_outer: bool = False, warmup: int = 0,
                 act_isub: int | None = None, dma_split: bool = False,
                 repeat: int = 1):
    absdiff = _register_absdiff()
    nc = bacc.Bacc("TRN2", target_bir_lowering=False, debug=False,
                   num_devices=NC)

    xT_d = nc.dram_tensor("xT", [F, W], bf16, kind="ExternalInput")
    kern_d = nc.dram_tensor("kern", [F, K * D], bf16, kind="ExternalInput")
    selw_d = nc.dram_tensor("selw", [128, 56], bf16, kind="ExternalInput")
    seljw_d = nc.dram_tensor("seljw", [128, 56], bf16, kind="ExternalInput")
    mi_d = nc.dram_tensor("mi_raw", [128, MY], f32, kind="ExternalOutput")
    mj_d = nc.dram_tensor("mj_raw", [128, JS1 - JS0], f32, kind="ExternalOutput")

    with tile.TileContext(nc) as tc:
        with tc.tile_pool(name="singles", bufs=1) as singles, \
             tc.tile_pool(name="ad", bufs=ad_bufs) as ad_pool, \
             tc.tile_pool(name="ep", bufs=e_bufs) as e_pool, \
             tc.tile_pool(name="mmps", bufs=l1_bufs, space="PSUM") as mm_pool, \
             tc.tile_pool(name="psjp", bufs=1, space="PSUM") as psj_pool, \
             tc.tile_pool(name="fill", bufs=2, space="PSUM") as fill_pool:

            # PE warmup: ~3.5us of throwaway matmuls on a memset tile so the
            # PE p-state governor reaches full clock before real work lands.
            if warmup:
                wsrc = singles.tile([128, 512], bf16)
                nc.vector.memset(wsrc[:], 0.0)
                wps = psj_pool.tile([32, 512], f32, tag="wps")
                for _ in range(warmup):
                    nc.tensor.matmul(wps[0:32, :], lhsT=wsrc[:, 0:32],
                                     rhs=wsrc[:], start=True, stop=True,
                                     skip_group_check=True,
                                     tile_position=(0, 0))

            kern_sb = singles.tile([128, 4, K * D], bf16)
            xT_sb = singles.tile([128, 4, W], bf16)
            if dma_split:
                # Peel the kg0 kern slices into small early DMAs so the
                # first mT matmul unblocks after ~128KB, then bulk-load the
                # rest. Interleave xT so rhs tiles land early too.
                for ft in range(4):
                    nc.sync.dma_start(
                        out=kern_sb[:, ft, 0:128],
                        in_=kern_d.ap()[ft * 128:(ft + 1) * 128, 0:128])
                    nc.sync.dma_start(out=xT_sb[:, ft, :],
                                      in_=xT_d.ap()[ft * 128:(ft + 1) * 128, :])
                for ft in range(4):
                    nc.sync.dma_start(
                        out=kern_sb[:, ft, 128:K * D],
                        in_=kern_d.ap()[ft * 128:(ft + 1) * 128, 128:K * D])
            else:
                for ft in range(4):
                    nc.sync.dma_start(out=kern_sb[:, ft, :],
                                      in_=kern_d.ap()[ft * 128:(ft + 1) * 128, :])
                    nc.sync.dma_start(out=xT_sb[:, ft, :],
                                      in_=xT_d.ap()[ft * 128:(ft + 1) * 128, :])
            selw = singles.tile([128, 56], bf16)
            nc.sync.dma_start(out=selw[:], in_=selw_d.ap())
            seljw = singles.tile([128, 56], bf16)
            nc.sync.dma_start(out=seljw[:], in_=seljw_d.ap())

            mt = singles.tile([128, KG, W], bf16)
            scol = singles.tile([128, KG, MY], f32)
            ncol = singles.tile([128, KG, MY], f32)
            mi_sb = singles.tile([128, MY], f32)
            mj_sb = singles.tile([128, JS1 - JS0], f32)

            def phase_a(kg):
                ps = mm_pool.tile([128, W], f32, tag="mmps")
                for ft in range(4):
                    nc.tensor.matmul(
                        ps[:],
                        lhsT=kern_sb[:, ft, 128 * kg:128 * (kg + 1)],
                        rhs=xT_sb[:, ft, :],
                        start=(ft == 0), stop=(ft == 3))
                nc.scalar.copy(mt[:, kg, :], ps[:])
                nc.vector.tensor_copy(scol[:, kg, :], mt[:, kg, 0:MY])
                nc.vector.tensor_scalar(ncol[:, kg, :], mt[:, kg, 0:MY],
                                        -1.0, None, mybir.AluOpType.mult)

            if not kg_outer:
                for kg in range(KG):
                    phase_a(kg)

            psj = psj_pool.tile([128, JS1 - JS0], f32)

            # Phase B: pairwise exp-L1.
            # Interleave DVE/ACT absdiff assignment so both engines feed PE
            # steadily: DVE gets n_dve of every 16, spread out.
            if act_isub is not None and n_dve == 15:
                dve_set = set(range(16)) - {act_isub}
            else:
                dve_set = set()
                acc = 0.0
                for isub in range(16):
                    acc += n_dve / 16.0
                    if acc >= 1.0 - 1e-9:
                        dve_set.add(isub)
                        acc -= 1.0
            mm_w = 4 if small_pe else W
            act_pos = sorted(set(range(16)) - dve_set)

            def emit_act_ads(ib, kg):
                tiles = {}
                for isub in act_pos:
                    t = ad_pool.tile([128, W], bf16, tag="act_ad")
                    nc.scalar.activation(
                        t[:], mt[:, kg, :], AF.Abs,
                        bias=ncol[:, kg, ib * 16 + isub:ib * 16 + isub + 1],
                        scale=1.0)
                    tiles[isub] = t
                return tiles

            def emit_jside(ib, kg, e):
                jq, jquad = kg % 4, kg // 4
                nc.tensor.matmul(
                    psj[32 * jquad:32 * jquad + 32, :],
                    lhsT=seljw[:, 24 - 8 * jq:56 - 8 * jq],
                    rhs=e[:, JS0:JS1],
                    start=(ib == 0 and jq == 0),
                    stop=(ib == IB - 1 and jq == 3),
                    skip_group_check=True,
                    tile_position=(0, 32 * jquad))

            glist = [(ib, kg) for ib in range(IB) for kg in range(KG)]
            if kg_outer:
                glist = [(ib, kg) for kg in range(KG) for ib in range(IB)]
                for kg in range(KG):
                    phase_a(kg)

            # Software-pipelined main loop: ACT absdiffs are emitted one
            # group ahead (so PE's first matmul never waits on an ACT op
            # queued behind the previous exp), and each group's j-side
            # matmul is delayed into the next group (so PE's in-order
            # queue doesn't stall on exp).
            def run_main():
                act_tiles = emit_act_ads(*glist[0])
                prev_e = None
                for g, (ib, kg) in enumerate(glist):
                    act_tiles, prev_e = run_group(g, ib, kg, act_tiles,
                                                  prev_e)
                emit_jside(*prev_e)

            def run_group(g, ib, kg, act_tiles, prev_e):
                l1 = mm_pool.tile([128, W], f32, tag="mmps")
                for isub in range(16):
                    i = ib * 16 + isub
                    if isub in dve_set:
                        ad = ad_pool.tile([128, W], bf16, tag="ad")
                        di = nc.vector._custom_dve(
                            absdiff, out=ad[:], in0=mt[:, kg, :],
                            s0=scol[:, kg, i:i + 1])
                        di.ins.perf_max = 3
                    else:
                        ad = act_tiles[isub]
                    q, quad = isub % 4, isub // 4
                    nc.tensor.matmul(
                        l1[32 * quad:32 * quad + 32, 0:mm_w],
                        lhsT=selw[:, 24 - 8 * q:56 - 8 * q],
                        rhs=ad[:, 0:mm_w],
                        start=(q == 0), stop=(q == 3),
                        skip_group_check=True,
                        tile_position=(0, 32 * quad))
                    if pe_filler:
                        fps = fill_pool.tile([32, pe_filler], f32, tag="fps")
                        nc.tensor.matmul(
                            fps[0:8, 0:pe_filler],
                            lhsT=selw[:, 24:32],
                            rhs=ad[:, 0:pe_filler],
                            start=True, stop=True,
                            skip_group_check=True,
                            tile_position=(0, 0))
                if g + 1 < len(glist):
                    act_tiles = emit_act_ads(*glist[g + 1])
                if prev_e is not None:
                    emit_jside(*prev_e)
                e = e_pool.tile([128, W], bf16, tag="e")
                col = ib * 16 + kg
                nc.scalar.activation(
                    e[:], l1[:], AF.Exp, scale=-1.0,
                    accum_out=mi_sb[:, col:col + 1])
                return act_tiles, (ib, kg, e)

            if repeat == 1:
                run_main()
            else:
                with tc.For_i(0, repeat, 1):
                    run_main()

            nc.vector.tensor_copy(mj_sb[:], psj[:])
            nc.sync.dma_start(out=mi_d.ap(), in_=mi_sb[:])
            nc.sync.dma_start(out=mj_d.ap(), in_=mj_sb[:])

    nc.compile()
    return nc


_NC_CACHE = None


def _get_module():
    global _NC_CACHE
    if _NC_CACHE is None:
        _NC_CACHE = build_module()
    return _NC_CACHE


def _host_inputs(x: np.ndarray, kernel: np.ndarray):
    xT = np.ascontiguousarray(x.T).astype(ml_dtypes.bfloat16)  # [F, B]
    kern = np.ascontiguousarray(kernel.reshape(F, K * D)).astype(
        ml_dtypes.bfloat16)
    selw = np.zeros((128, 56), dtype=ml_dtypes.bfloat16)
    seljw = np.zeros((128, 56), dtype=ml_dtypes.bfloat16)
    for p in range(128):
        selw[p, 24 + (p >> 4)] = 1.0    # p = k_sub*16 + d -> k_sub
        seljw[p, 24 + (p & 7)] = 1.0    # p = i_sub*8 + k_sub -> k_sub
    in_maps = []
    for d in range(NC):
        cols = (64 * d + np.arange(W)) % B
        in_maps.append({
            "xT": np.ascontiguousarray(xT[:, cols]),
            "kern": kern,
            "selw": selw,
            "seljw": seljw,
        })
    return in_maps


def _gather(results, x: np.ndarray) -> np.ndarray:
    mb = np.zeros((B, K), np.float32)
    for d in range(NC):
        mi = results[d]["mi_raw"]                 # [128, 64]
        M = mi.reshape(16, 8, IB, KG)             # [i_sub, k_sub, ib, kg]
        mb[64 * d:64 * d + MY, :] += M.transpose(2, 0, 3, 1).reshape(MY, K)
        cols = (64 * d + JS0 + np.arange(JS1 - JS0)) % B
        mb[cols, :] += results[d]["mj_raw"].T     # [192, 128]
    return np.concatenate([x.astype(np.float32), mb], axis=1)


def kernel(x: np.ndarray, kernel: np.ndarray) -> np.ndarray:
    x = np.asarray(x)
    kernel = np.asarray(kernel)
    nc = _get_module()
    in_maps = _host_inputs(x, kernel)
    res = run_bass_kernel_spmd(nc, in_maps, list(range(NC)))
    return _gather(res.results, x)
